# revision 76
# baseline (speedup 1.0000x reference)
"""Cross-attention Trainium2 Bass kernel (fp8 DoubleRow, software-pipelined).

Computes: out = softmax((x@Wq) @ (ctx@Wk)^T / sqrt(D)) @ (ctx@Wv) + x
for x:[B,N,D]=(4,4096,512), ctx:[B,M,C]=(4,4096,768).

Sharding: 8 cores = (batch b in 0..3) x (query-half h in 0..1). Each core
handles 2048 queries against its batch's full 4096-key context. Pure SPMD,
no collectives.

Host prep: shard, transpose to d-major, cast (xT fp8e4, x natural bf16,
ctxT fp8, weights fp8), pack to [128, ...] partition-major so every
tensor loads in one (or few) large DMAs (HWDGE costs ~625ns per DMA
instruction), and fold the Q projection into the K-side weight
(W_kq = Wk @ Wq^T, computed once in f32): S = q k^T = x (ctx W_kq)^T,
so the kernel's score matmuls consume x^T directly. Output is written
in natural [q, d] layout as bf16 (halves store traffic; ~0.06% extra
rounding on a 2% budget); host upcasts and unpacks the q-tile dim.

Device math, all matmuls fp8e4 DoubleRow (2 k-tiles per instruction,
0.5 cyc/row):
  - projections K'^T (d-major, pre-folded weight) and V (key-major)
    accumulate in PSUM pair tiles, evacuated as single [128,1024]
    copies to fp8 SBUF (alternating ACT/DVE)
  - attention per 512-query chunk: per key-tile pair, S^T pair in PSUM,
    one exp -> fp8 P8 (ACT); O accumulates in NATURAL [q,d] layout
    (lhsT = P8 slice stationary, V moving) so the softmax normalizer is a
    per-partition scalar: out = (O * (1/L)[q]) + x fuses into ONE
    scalar_tensor_tensor per q-tile
  - denominator L^T accumulates directly in query-partition layout via
    ~free K=1 DoubleRow matmuls against a ones column (one per q-tile per
    key pair); per-q-tile reciprocals on DVE feed the fused epilogue
  - O splits into pass-A (q-tile 0, accumulated inline) and pass-B
    (q-tiles 1..3, deferred): each chunk's tail (reciprocals, pass-B,
    fused normalize+residual, stores) is spread piecewise through the
    NEXT chunk's score/exp stream so PE and ACT never drain; projections
    are spread through chunk 0 the same way; the last chunk's tail
    overlaps pass-B on PE with the DVE normalize chain.
PSUM: score-pair ring 3 x [128,2,512] (6 banks) + O pass-A (1 bank) +
L accumulator (1 bank) = exactly 8 banks.
"""
import sys


def _ensure_concourse():
    try:
        import concourse  # noqa: F401
    except ImportError:
        for p in ("/opt/trn_rl_repo", "/root/.axon_site/_ro/trn_rl_repo"):
            if p not in sys.path:
                sys.path.insert(0, p)


_ensure_concourse()

import numpy as np
import ml_dtypes

import concourse.bacc as bacc
import concourse.tile as tile
from concourse import mybir
from concourse.bass_utils import run_bass_kernel_spmd

F32 = mybir.dt.float32
BF16 = mybir.dt.bfloat16
F8 = mybir.dt.float8e4
NP_F8 = ml_dtypes.float8_e4m3
NP_BF16 = ml_dtypes.bfloat16
DR = mybir.MatmulPerfMode.DoubleRow
ALU = mybir.AluOpType

DIM = 512
CTX = 768
B, N, M = 4, 4096, 4096
NCORES = 8
QCH = 512
SCALE = float(DIM) ** -0.5

N_DT = DIM // 128   # 4 d tiles
N_CT = CTX // 128   # 6 c tiles


def build_nc(n_q, n_keys):
    """Per-core SPMD program: n_q queries x n_keys context rows."""
    assert n_q % QCH == 0 and n_keys % 1024 == 0
    n_qch = n_q // QCH        # query chunks (4)
    n_kc = n_keys // 512      # key chunks (8)
    n_ktp = n_keys // 256     # key-tile pairs (16)
    n_qt = n_q // 128         # query tiles (16)

    nc = bacc.Bacc(None, target_bir_lowering=False)

    x16_d = nc.dram_tensor("x16", [128, n_qt, DIM], BF16, kind="ExternalInput")
    xT8_d = nc.dram_tensor("xT8", [128, N_DT, n_q], F8, kind="ExternalInput")
    ctx8_d = nc.dram_tensor("ctxT8", [128, N_CT, n_keys], F8, kind="ExternalInput")
    # wk8 holds the folded W_kq = Wk @ Wq^T (host weight prep), so the
    # score matmuls consume xT8 directly and the Q projection disappears
    wk8_d = nc.dram_tensor("wk8", [128, N_CT, DIM], F8, kind="ExternalInput")
    wv8_d = nc.dram_tensor("wv8", [128, N_CT, DIM], F8, kind="ExternalInput")
    out_d = nc.dram_tensor("out", [128, n_qt, DIM], BF16, kind="ExternalOutput")

    ones8_d = nc.inline_tensor(np.ones((128, 2), NP_F8), "ones8")

    with tile.TileContext(nc) as tc:
        with (
            tc.tile_pool(name="const", bufs=1) as const,
            tc.tile_pool(name="res", bufs=1) as res,
            tc.tile_pool(name="p8", bufs=2) as p8_pool,
            tc.tile_pool(name="fin", bufs=2) as fin,
            tc.tile_pool(name="sc", bufs=3, space="PSUM") as sc,
            tc.tile_pool(name="opool", bufs=1, space="PSUM") as opool,
            tc.tile_pool(name="lpool", bufs=1, space="PSUM") as lpool,
        ):
            ones8 = const.tile([128, 2, 1], F8)
            wk8 = res.tile([128, N_CT, DIM], F8)
            wv8 = res.tile([128, N_CT, DIM], F8)
            XT8 = res.tile([128, N_DT, n_q], F8)
            X16 = res.tile([128, n_qt, DIM], BF16)
            CT8 = res.tile([128, N_CT, n_keys], F8)
            KT8 = res.tile([128, N_DT, n_keys], F8)
            V8 = res.tile([128, n_keys // 128, DIM], F8)

            # ---- PE clock warm-up: dummy matmuls on memset data keep the
            # tensor engine "continuously executing" through the DMA ramp so
            # real work starts at full pstate ----
            warm = const.tile([128, 512], BF16)
            nc.vector.memset(warm, 1.0)
            wps = sc.tile([128, 2, QCH], F32, tag="sc", name="warmps")
            for w in range(10):
                nc.tensor.matmul(wps[0:16, 0, :], lhsT=warm[:, 0:16],
                                 rhs=warm[:, :], start=True, stop=True)

            # ---- input DMAs: few and large; ordered by first use ----
            nc.sync.dma_start(out=wv8[:, 0:2, :], in_=wv8_d[:, 0:2, :])
            nc.sync.dma_start(out=CT8[:, 0:2, 0:512], in_=ctx8_d[:, 0:2, 0:512])
            nc.sync.dma_start(out=wv8[:, 2:4, :], in_=wv8_d[:, 2:4, :])
            nc.sync.dma_start(out=CT8[:, 2:4, 0:512], in_=ctx8_d[:, 2:4, 0:512])
            nc.sync.dma_start(out=wv8[:, 4:6, :], in_=wv8_d[:, 4:6, :])
            nc.sync.dma_start(out=CT8[:, 4:6, 0:512], in_=ctx8_d[:, 4:6, 0:512])
            nc.sync.dma_start(out=wk8, in_=wk8_d[:])
            nc.sync.dma_start(out=CT8[:, :, 512:1024],
                              in_=ctx8_d[:, :, 512:1024])
            nc.sync.dma_start(out=XT8[:, :, 0:QCH], in_=xT8_d[:, :, 0:QCH])
            nc.sync.dma_start(out=ones8, in_=ones8_d[:])
            nc.sync.dma_start(out=XT8[:, :, QCH:n_q], in_=xT8_d[:, :, QCH:n_q])
            nc.sync.dma_start(out=CT8[:, :, 1024:1536],
                              in_=ctx8_d[:, :, 1024:1536])
            nc.sync.dma_start(out=CT8[:, :, 1536:2048],
                              in_=ctx8_d[:, :, 1536:2048])
            nc.sync.dma_start(out=CT8[:, :, 2048:2560],
                              in_=ctx8_d[:, :, 2048:2560])
            nc.sync.dma_start(out=CT8[:, :, 2560:3072],
                              in_=ctx8_d[:, :, 2560:3072])
            nc.sync.dma_start(out=CT8[:, :, 3072:3584],
                              in_=ctx8_d[:, :, 3072:3584])
            nc.sync.dma_start(out=CT8[:, :, 3584:4096],
                              in_=ctx8_d[:, :, 3584:4096])
            nc.sync.dma_start(out=X16, in_=x16_d[:])

            # ---- projection pieces (fp8 DoubleRow; pair-tile psum) ----
            evac_ctr = [0]

            def evac(dst, ps):
                if evac_ctr[0] % 2 == 0:
                    nc.scalar.copy(out=dst, in_=ps)
                else:
                    nc.vector.tensor_copy(out=dst, in_=ps)
                evac_ctr[0] += 1

            def vproj_half(kc, half):
                kt0 = kc * 4 + half * 2
                ps = sc.tile([128, 2, DIM], F32, tag="sc",
                             name=f"psv{kc}_{half}")
                for j in (0, 1):
                    kt = kt0 + j
                    for t in (0, 1, 2):
                        nc.tensor.matmul(
                            ps[:, j, :],
                            lhsT=CT8[:, 2 * t:2 * t + 2,
                                     kt * 128:(kt + 1) * 128],
                            rhs=wv8[:, 2 * t:2 * t + 2, :],
                            start=(t == 0), stop=(t == 2), perf_mode=DR)
                evac(V8[:, kt0:kt0 + 2, :], ps)

            def kproj_half(kc, dtp):
                ksl = slice(kc * 512, (kc + 1) * 512)
                ps = sc.tile([128, 2, 512], F32, tag="sc",
                             name=f"psk{kc}_{dtp}")
                for j in (0, 1):
                    dt = 2 * dtp + j
                    for t in (0, 1, 2):
                        nc.tensor.matmul(
                            ps[:, j, :],
                            lhsT=wk8[:, 2 * t:2 * t + 2,
                                     dt * 128:(dt + 1) * 128],
                            rhs=CT8[:, 2 * t:2 * t + 2, ksl],
                            start=(t == 0), stop=(t == 2), perf_mode=DR)
                evac(KT8[:, 2 * dtp:2 * dtp + 2, ksl], ps)

            # ---- attention pieces ----
            st = {}   # qc -> tiles

            def attn_begin(qc):
                st[qc] = {
                    "P8t": p8_pool.tile([128, n_ktp, 2, QCH], F8, tag="p8",
                                        name=f"p8_{qc}"),
                    "o_a": opool.tile([128, 1, QCH], F32, tag="o",
                                      name=f"oa{qc}"),
                    "l_n": lpool.tile([128, QCH], F32, tag="l",
                                      name=f"ln{qc}"),
                }
                st[qc]["srcmap"] = {0: st[qc]["o_a"][:, 0, :]}

            def emit_s_exp(qc, ktp):
                d = st[qc]
                qsl = slice(qc * QCH, (qc + 1) * QCH)
                s_t = sc.tile([128, 2, QCH], F32, tag="sc",
                              name=f"s{qc}_{ktp}")
                for j in (0, 1):
                    kt = 2 * ktp + j
                    for dtp in (0, 1):
                        nc.tensor.matmul(
                            s_t[:, j, :],
                            lhsT=KT8[:, 2 * dtp:2 * dtp + 2,
                                     kt * 128:(kt + 1) * 128],
                            rhs=XT8[:, 2 * dtp:2 * dtp + 2, qsl],
                            start=(dtp == 0), stop=(dtp == 1), perf_mode=DR)
                nc.scalar.activation(
                    out=d["P8t"][:, ktp, :, :], in_=s_t[:, :, :],
                    func=mybir.ActivationFunctionType.Exp, scale=SCALE)

            def o_mm(qc, ktp, qt, dst):
                # natural-layout O: lhsT = P8 slice (stationary), V moving
                d = st[qc]
                nc.tensor.matmul(
                    dst,
                    lhsT=d["P8t"][:, ktp, :, qt * 128:(qt + 1) * 128],
                    rhs=V8[:, 2 * ktp:2 * ktp + 2, :],
                    start=(ktp == 0), stop=(ktp == n_ktp - 1),
                    perf_mode=DR)

            def emit_oa(qc, ktp):
                # pass-A O for q-tile 0 plus the ~free L tinies:
                # L^T[q] accumulates via K=1 DoubleRow against a ones column
                d = st[qc]
                o_mm(qc, ktp, 0, d["o_a"][:, 0, :])
                for qt in range(4):
                    nc.tensor.matmul(
                        d["l_n"][:, qt * 128:qt * 128 + 1],
                        lhsT=d["P8t"][:, ktp, :, qt * 128:(qt + 1) * 128],
                        rhs=ones8,
                        start=(ktp == 0), stop=(ktp == n_ktp - 1),
                        perf_mode=DR)

            def recip_scalars(qc):
                d = st[qc]
                d["rTs"] = fin.tile([128, 4], F32, tag="rTs",
                                    name=f"rTs{qc}")
                for qt in range(4):
                    nc.vector.reciprocal(
                        out=d["rTs"][:, qt:qt + 1],
                        in_=d["l_n"][:, qt * 128:qt * 128 + 1])
                d["ob"] = fin.tile([128, 4, DIM], BF16, tag="ob",
                                   name=f"obs{qc}")

            def ob_alloc(qc, qt, from_sc=False):
                if from_sc:
                    st[qc][f"o_b{qt}"] = sc.tile([128, 2, QCH], F32,
                                                 tag="sc", name=f"obp{qc}_{qt}")
                else:
                    st[qc][f"o_b{qt}"] = opool.tile([128, 1, QCH], F32,
                                                    tag="o", name=f"obp{qc}_{qt}")

            def passB_block(qc, qt, ktps, bank=0):
                d = st[qc]
                for ktp in ktps:
                    o_mm(qc, ktp, qt, d[f"o_b{qt}"][:, bank, :])

            def stt(qc, qt):
                # out = O[qt] * (1/L)[q] + x   — one fused DVE op
                d = st[qc]
                src = d["srcmap"][qt]
                nc.vector.scalar_tensor_tensor(
                    out=d["ob"][:, qt, :], in0=src,
                    scalar=d["rTs"][:, qt:qt + 1],
                    in1=X16[:, qc * 4 + qt, :],
                    op0=ALU.mult, op1=ALU.add)

            def store(qc, qts):
                d = st[qc]
                g0 = qc * 4 + qts[0]
                nc.sync.dma_start(
                    out=out_d[:, g0:g0 + len(qts), :],
                    in_=d["ob"][:, qts[0]:qts[0] + len(qts), :])

            # ---- schedule ----
            # chunk 0: projections spread through the score/exp stream
            attn_begin(0)
            for kc in (0, 1):
                vproj_half(kc, 0); vproj_half(kc, 1)
                kproj_half(kc, 0); kproj_half(kc, 1)
            for t in range(n_ktp):
                emit_s_exp(0, t)
                emit_oa(0, t)
                kc = t + 2
                if kc < n_kc:
                    vproj_half(kc, 0); vproj_half(kc, 1)
                    kproj_half(kc, 0); kproj_half(kc, 1)

            # chunks 1..n-1: previous chunk's tail spread through the
            # stream; pass-B runs q-tiles 1..3 through the 1-slot O ring
            def set_src(qc, qt):
                st[qc]["srcmap"][qt] = st[qc][f"o_b{qt}"][:, 0, :]

            for qc in range(1, n_qch):
                p = qc - 1
                attn_begin(qc)
                fillers = [
                    lambda: recip_scalars(p),
                    lambda: stt(p, 0),
                    lambda: (ob_alloc(p, 1), set_src(p, 1),
                             passB_block(p, 1, range(0, 8))),
                    lambda: passB_block(p, 1, range(8, 16)),
                    lambda: stt(p, 1),
                    lambda: (ob_alloc(p, 2), set_src(p, 2),
                             passB_block(p, 2, range(0, 8))),
                    lambda: (passB_block(p, 2, range(8, 16)),
                             store(p, (0, 1))),
                    lambda: stt(p, 2),
                    lambda: (ob_alloc(p, 3), set_src(p, 3),
                             passB_block(p, 3, range(0, 8))),
                    lambda: (passB_block(p, 3, range(8, 16)),
                             store(p, (2,))),
                    lambda: (stt(p, 3), store(p, (3,))),
                ]
                last = (qc == n_qch - 1)
                oa_backlog = []
                for ktp in range(n_ktp):
                    emit_s_exp(qc, ktp)
                    if last and ktp == n_ktp - 1:
                        # alloc after S15: lands on a drained slot, displaces
                        # no score allocation
                        ob_alloc(qc, 1, from_sc=True)
                        st[qc]["srcmap"][1] = st[qc]["o_b1"][:, 0, :]
                        st[qc]["srcmap"][2] = st[qc]["o_b1"][:, 1, :]
                        st[qc]["o_b2"] = st[qc]["o_b1"]
                    if fillers:
                        fillers.pop(0)()
                        oa_backlog.append(ktp)
                    else:
                        if oa_backlog:
                            # drain the deferred pass-A work a few per unit
                            oa_backlog.append(ktp)
                            take = min(len(oa_backlog),
                                       max(3, -(-len(oa_backlog) //
                                                max(1, n_ktp - ktp))))
                            for k2 in oa_backlog[:take]:
                                emit_oa(qc, k2)
                            oa_backlog = oa_backlog[take:]
                        else:
                            emit_oa(qc, ktp)
                    if last and ktp == n_ktp - 1:
                        # half of q-tile 1's pass-B runs during the last exp
                        passB_block(qc, 1, range(0, 8))
                st.pop(p)

            # final chunk tail: q-tiles 1,2 accumulate in a borrowed score
            # slot, q-tile 3 in the O ring; stores drain per q-tile
            p = n_qch - 1
            recip_scalars(p)
            stt(p, 0)
            store(p, (0,))
            passB_block(p, 1, range(8, n_ktp))
            ob_alloc(p, 3)
            set_src(p, 3)
            passB_block(p, 2, range(0, n_ktp), bank=1)
            stt(p, 1)
            store(p, (1,))
            passB_block(p, 3, range(0, n_ktp))
            stt(p, 2)
            store(p, (2,))
            stt(p, 3)
            store(p, (3,))
            st.pop(p)

    nc.finalize()
    return nc


SHARD_SHAPE = (N // 2, M)   # (n_q, n_keys) per core

_NC_CACHE = {}


def _get_nc(n_q, n_keys):
    key = (n_q, n_keys)
    if key not in _NC_CACHE:
        _NC_CACHE[key] = build_nc(n_q, n_keys)
    return _NC_CACHE[key]


def _pack(a, nt):
    """[nt*128, F] -> [128, nt, F] partition-major."""
    return np.ascontiguousarray(
        a.reshape(nt, 128, a.shape[1]).transpose(1, 0, 2))


def shard_inputs(x, context, Wq, Wk, Wv):
    """8 shards: (batch, query-half). Host-side layout prep only."""
    n_q = N // 2
    # weight folding: S = q k^T = x (Wk Wq^T applied to ctx)^T, so the
    # Q projection folds into the K-side weight (computed once, f32)
    wkq = (Wk.astype(np.float32) @ Wq.astype(np.float32).T)
    wk8 = _pack(wkq.astype(NP_F8), N_CT)
    wv8 = _pack(Wv.astype(NP_F8), N_CT)
    in_maps = []
    for core in range(NCORES):
        b, h = divmod(core, 2)
        xs = x[b, h * n_q:(h + 1) * n_q, :]
        xT = np.ascontiguousarray(xs.T)
        ctxT = np.ascontiguousarray(context[b].T)
        in_maps.append({
            "x16": _pack(xs.astype(NP_BF16), n_q // 128),
            "xT8": _pack(xT.astype(NP_F8), N_DT),
            "ctxT8": _pack(ctxT.astype(NP_F8), N_CT),
            "wk8": wk8, "wv8": wv8,
        })
    return in_maps


def unshard_output(results):
    n_q = N // 2
    out = np.empty((B, N, DIM), np.float32)
    for core in range(NCORES):
        b, h = divmod(core, 2)
        o = results[core]["out"]          # [128, n_qt, DIM] bf16
        out[b, h * n_q:(h + 1) * n_q, :] = (
            o.astype(np.float32).transpose(1, 0, 2).reshape(n_q, DIM))
    return out


def kernel(x, context, Wq, Wk, Wv):
    x = np.asarray(x, np.float32)
    context = np.asarray(context, np.float32)
    Wq = np.asarray(Wq, np.float32)
    Wk = np.asarray(Wk, np.float32)
    Wv = np.asarray(Wv, np.float32)
    nc = _get_nc(N // 2, M)
    in_maps = shard_inputs(x, context, Wq, Wk, Wv)
    res = run_bass_kernel_spmd(nc, in_maps, list(range(NCORES)))
    return unshard_output(res.results)
